# revision 5
# baseline (speedup 1.0000x reference)
"""GAT (3-layer, PyG GATConv semantics) on 8 Trainium2 NeuronCores.

Strategy (dst-node sharding):
- Nodes padded to 50176 = 8 * 6272; core c owns dst rows [c*6272, (c+1)*6272).
- Per layer:
  dense (sharded): table_shard = h_shard @ [W*bn_a | W@a_src | W@a_dst] in bf16
      (bn scale folded into W's feature columns on the host), written into the
      chunk-major table chunks that feed the AllGathers; the core's own rows
      are read back from those chunks (no separate own-table copy).
  AllGather the padded table so every core can fetch arbitrary src rows.
      Collectives are issued on the SCALAR engine so they do not head-of-line
      block the gpsimd gather stream (collectives hold the issuing engine
      until completion on HW).
  edge phase: edges bucketed by dst into 128-slot blocks, with per-block
      exact sub-tile counts (nlo_b/nhi_b = max over cores), compiled into the
      program (kernel.py compiles after seeing the edge structure). Self-loops
      are NOT gathered: their contribution comes from the local table rows via
      an identity-lhsT matmul. Remaining edges are packed lo/hi (chunk-0 /
      chunk-1 of the gathered table; int16 index range), padded with trailing
      -1 indices; the true per-block descriptor count is passed at runtime via
      reg_load from an SBUF counts tile (8 rotating registers to avoid a
      write-after-read serialization chain on the registers).
      Per block: batched dma_gather of src rows, one-hot selection matrices U
      (edges-on-partitions) AND U^T (slots-on-partitions) both built directly
      with is_equal from host-prepared dst-slot tables -- no PE transposes.
      d[dst] per edge = U^T-lhsT matmul against the block's local d rows.
      ex = Exp(Lrelu(s_src + d_dst)) fused on ACT (2 ops, no DVE max).
      ex-weighted features + ex accumulated into PSUM with one matmul per
      128-edge sub-tile, then normalize / bn / activation fused on DVE+ACT.
"""
import os
import sys
import types

sys.path.insert(0, "/opt/trn_rl_repo")

import numpy as np
import ml_dtypes


def _install_ntff_shim():
    """Provide antenv.axon_hooks so run_bass_kernel_spmd(trace=True) works."""
    try:
        import antenv

        if "antenv.axon_hooks" in sys.modules:
            return
        mod = types.ModuleType("antenv.axon_hooks")
        mod._hook = None
        mod.set_axon_ntff_profile_hook = lambda h: setattr(mod, "_hook", h)
        mod.get_axon_ntff_profile_hook = lambda: mod._hook
        sys.modules["antenv.axon_hooks"] = mod
        antenv.axon_hooks = mod
        from trn_agent_boot.trn_boot import _ntff_profile_via_ctypes

        hook = _ntff_profile_via_ctypes("/opt/axon/libaxon_pjrt.so")
        if hook is not None:
            mod.set_axon_ntff_profile_hook(hook)
    except Exception:
        pass


_install_ntff_shim()

import concourse.bass as bass
import concourse.bacc as bacc
import concourse.mybir as mybir
import concourse.tile as tile
from concourse.bass_utils import run_bass_kernel_spmd
from concourse.masks import make_identity

bfnp = ml_dtypes.bfloat16
f32 = mybir.dt.float32
bf16 = mybir.dt.bfloat16
i16 = mybir.dt.int16
i32 = mybir.dt.int32
AF = mybir.ActivationFunctionType
OP = mybir.AluOpType

N, E = 50000, 800000
DIN, HID, HEADS, DOUT = 128, 32, 8, 16
NEG = 0.2
EPS = 1e-5

NCORES = 8
SHARD = 6272
NPAD = NCORES * SHARD  # 50176
NBLK = SHARD // 128  # 49

# table row pitches (bf16 cols; byte pitch must be a multiple of 256)
DROW12, USED12 = 384, 272  # [xw(256) | s(8) | d(8) | pad]
DROW3, USED3 = 128, 18  # [xw(16) | s(1) | d(1) | pad]
PAD_DSTL = 200.0  # one-hot miss marker for padding edges
GBUFS = 5
SINGLE_PACKET = False
NREGS = 8

# The full table is laid out chunk-major: chunk 0 holds every core's shard
# rows [0, CH0), chunk 1 the rest. Each chunk is then a CONTIGUOUS AllGather
# destination; chunk 0's collective starts while the tail of the previous
# edge phase still runs.
CH0_BLKS = 31
CH0 = CH0_BLKS * 128  # 3968
CH1 = SHARD - CH0  # 2304
POS0 = NCORES * CH0  # 31744 (< 32768 so chunk-0 positions fit int16)


def _pos(g):
    """global node id -> chunk-major position in the gathered table."""
    c, r = g // SHARD, g % SHARD
    return np.where(r < CH0, c * CH0 + r, POS0 + c * CH1 + (r - CH0))


def _build_Wp(W, a_s, a_d, bn_a):
    H, F = a_s.shape
    Ws = np.stack([W[:, h * F : (h + 1) * F] @ a_s[h] for h in range(H)], axis=1)
    Wd = np.stack([W[:, h * F : (h + 1) * F] @ a_d[h] for h in range(H)], axis=1)
    return np.concatenate([W * bn_a[None, :], Ws, Wd], axis=1).astype(bfnp)


def _wrap_idx(flat):
    """int16 list -> [128, len/16] wrapped in 16 partitions, replicated x8."""
    n = len(flat)
    assert n % 16 == 0
    w = flat.reshape(n // 16, 16).T  # [16, n/16]
    return np.tile(w, (8, 1)).astype(np.int16)


def _prep_edges(edge_src, edge_dst):
    """Bucket non-self-loop edges by (core, block). Per-block sub-tile
    capacities nlo_b/nhi_b are the max over cores (the SPMD program is shared
    across cores). Returns per-core packed meta (idx_lo | idx_hi | dstl),
    per-core dstlT rows, per-block valid counts, and the (nlo, nhi) lists."""
    src = _pos(edge_src.astype(np.int64))
    dst = edge_dst.astype(np.int64)
    core = dst // SHARD
    blk = (dst % SHARD) // 128

    per_block = {}
    for c in range(NCORES):
        m = core == c
        for b in range(NBLK):
            mb = m & (blk == b)
            s_all, d_all = src[mb], dst[mb]
            order = np.argsort(s_all, kind="stable")
            per_block[(c, b)] = (s_all[order], d_all[order])

    nlo = np.zeros(NBLK, np.int64)
    nhi = np.zeros(NBLK, np.int64)
    for (c, b), (s_all, _) in per_block.items():
        lo = int((s_all < POS0).sum())
        hi = len(s_all) - lo
        nlo[b] = max(nlo[b], (max(lo, 1) + 127) // 128)
        nhi[b] = max(nhi[b], (max(hi, 1) + 127) // 128)
    nsub = nlo + nhi
    ml = nlo * 8 + nhi * 8 + nsub
    ML = int(ml.max())
    NSUBMAX = int(nsub.max())

    out = []
    for c in range(NCORES):
        meta = np.zeros((NBLK, 128, ML), np.int16)
        meta2 = np.zeros((NBLK, 1, NSUBMAX * 128), np.float32)
        counts = np.zeros((NBLK, 2), np.int32)
        for b in range(NBLK):
            s_all, d_all = per_block[(c, b)]
            ncut = int((s_all < POS0).sum())
            lo_s, lo_d = s_all[:ncut], d_all[:ncut]
            hi_s, hi_d = s_all[ncut:] - POS0, d_all[ncut:]
            NLO, NHI = int(nlo[b]), int(nhi[b])
            NSUB = NLO + NHI
            dstl = np.full((NSUB * 128,), PAD_DSTL, np.float32)
            for half, (ss, dd, cap, off) in enumerate(
                ((lo_s, lo_d, NLO, 0), (hi_s, hi_d, NHI, NLO))
            ):
                n = len(ss)
                assert n <= cap * 128, (c, b, half, n)
                sp = np.full(cap * 128, -1, np.int16)
                sp[:n] = ss.astype(np.int16)
                dl = dstl[off * 128 : (off + cap) * 128]
                dl[:n] = (dd - c * SHARD - b * 128).astype(np.float32)
                if n == 0:  # keep the gather ucode's count >= 1
                    sp[0], n = 0, 1
                counts[b, half] = n
                w = _wrap_idx(sp)
                if half == 0:
                    meta[b, :, 0 : NLO * 8] = w
                else:
                    meta[b, :, NLO * 8 : NLO * 8 + NHI * 8] = w
            # gathered row i lands at [p=i%128, j=i//128]
            dst_slot = dstl.reshape(NSUB, 128).T.astype(bfnp)  # [128, NSUB]
            meta[b, :, NLO * 8 + NHI * 8 : NLO * 8 + NHI * 8 + NSUB] = (
                dst_slot.view(np.int16)
            )
            # transposed layout for the U^T build: [j, e] flattened
            meta2[b, 0, 0 : NSUB * 128] = dstl
        out.append(
            {
                "meta": meta,
                "meta2": meta2.astype(bfnp).view(np.int16),
                "counts": counts.reshape(-1),
            }
        )
    return out, [int(v) for v in nlo], [int(v) for v in nhi], ML, NSUBMAX


def _dense_phase(nc, tc, sb, ps, h_in, w_tiles, tb_c0, tb_c1, used, h_cols):
    """rows = h_in @ W' written into the padded chunk-major table halves
    tb_c0/tb_c1 that feed the AllGathers (the core's own rows are read back
    from these chunks in the edge phase). h_in is a DRAM [SHARD, h_cols] bf16
    tensor (row-major). Transpose-loads and table writes are batched 4 blocks
    at a time; batches never straddle the chunk boundary."""
    nk = h_cols // 128
    BB = 4
    t = 0
    while t < NBLK:
        lim = CH0_BLKS if t < CH0_BLKS else NBLK
        nb = min(BB, lim - t)
        hts = []
        for k in range(nk):
            ht = sb.tile([128, nb * 128], bf16, tag="ht", bufs=2, name=f"ht{k}")
            nc.sync.dma_start(
                out=ht[:],
                in_=h_in[t * 128 : (t + nb) * 128, k * 128 : (k + 1) * 128],
                transpose=True,
            )
            hts.append(ht)
        rows = sb.tile([128, nb, used], bf16, tag="drow", bufs=2)
        for j in range(nb):
            psd = ps.tile([128, used], f32, tag="psd", bufs=2)
            for k in range(nk):
                nc.tensor.matmul(
                    out=psd[:],
                    lhsT=hts[k][:, j * 128 : (j + 1) * 128],
                    rhs=w_tiles[k][:],
                    start=(k == 0),
                    stop=(k == nk - 1),
                )
            nc.scalar.activation(out=rows[:, j, :], in_=psd[:], func=AF.Copy)
        if t < CH0_BLKS:
            tb_c, r0 = tb_c0, t * 128
        else:
            tb_c, r0 = tb_c1, t * 128 - CH0
        nc.sync.dma_start(
            out=tb_c[r0 : r0 + nb * 128, 0:used].rearrange(
                "(j p) c -> p j c", j=nb
            ),
            in_=rows[:, 0:nb, :],
        )
        t += nb


def _edge_phase(
    nc,
    tc,
    sb,
    ps,
    gbufs,
    table_c0,
    table_c1,
    own_c0,
    own_c1,
    t_meta,
    t_meta2,
    counts_t,
    regs,
    iota_t,
    iota_p,
    ident_t,
    H,
    F,
    drow,
    used,
    bn_c_t,
    h_out,
    final,
    nlo_l,
    nhi_l,
    NSUBMAX,
    ag_c1=None,
):
    HF = H * F
    rcols = HF + H  # matmul rhs cols: [gw | ex]

    K = GBUFS - 1  # lo-gathers run K blocks ahead of hi-gathers
    metas = {}

    ML_MAX = t_meta.shape[2]

    def issue_lo(b):
        NLO, NHI = nlo_l[b], nhi_l[b]
        NSUB = NLO + NHI
        ML = NLO * 8 + NHI * 8 + NSUB
        meta = sb.tile([128, ML_MAX], i16, tag="meta", bufs=K + 3)
        nc.sync.dma_start(out=meta[:, 0:ML], in_=t_meta[b, :, 0:ML])
        metas[b] = meta
        r = regs[(2 * b) % NREGS]
        nc.gpsimd.reg_load(r, counts_t[0:1, 2 * b : 2 * b + 1])
        nc.gpsimd.dma_gather(
            out_ap=gbufs[b % GBUFS][:, 0:NLO, :],
            in_ap=table_c0[:],
            idxs_ap=meta[:16, 0 : NLO * 8],
            num_idxs=NLO * 128,
            num_idxs_reg=r,
            elem_size=drow,
            single_packet=SINGLE_PACKET,
            queue_num=(2 * b) % 4,
        )

    for b in range(K):
        issue_lo(b)
    if ag_c1 is not None:
        ag_c1()
    for b in range(NBLK):
        NLO, NHI = nlo_l[b], nhi_l[b]
        NSUB = NLO + NHI
        if b + K < NBLK:
            issue_lo(b + K)
        meta = metas.pop(b)
        own = sb.tile([128, used], bf16, tag="own", bufs=4)
        if b < CH0_BLKS:
            nc.sync.dma_start(
                out=own[:], in_=own_c0[b * 128 : (b + 1) * 128, 0:used]
            )
        else:
            r0 = b * 128 - CH0
            nc.sync.dma_start(out=own[:], in_=own_c1[r0 : r0 + 128, 0:used])
        dstl = meta[:, NLO * 8 + NHI * 8 : NLO * 8 + NHI * 8 + NSUB].bitcast(bf16)

        rh = regs[(2 * b + 1) % NREGS]
        nc.gpsimd.reg_load(rh, counts_t[0:1, 2 * b + 1 : 2 * b + 2])

        # pinned, pre-zeroed buffers: slots skipped by the runtime descriptor
        # count only ever expose older gathered rows (finite), never raw SBUF.
        G = gbufs[b % GBUFS]
        nc.gpsimd.dma_gather(
            out_ap=G[:, NLO:NSUB, :],
            in_ap=table_c1[:],
            idxs_ap=meta[:16, NLO * 8 : NLO * 8 + NHI * 8],
            num_idxs=NHI * 128,
            num_idxs_reg=rh,
            elem_size=drow,
            single_packet=SINGLE_PACKET,
            queue_num=(2 * b + 1) % 4,
        )

        # one-hot selection matrices, U[e, j, slot] = (dst_local[e, j] == slot)
        Uf = sb.tile([128, NSUBMAX, 128], bf16, tag="U", bufs=3)
        U = Uf[:, 0:NSUB, :]
        nc.vector.tensor_tensor(
            out=U,
            in0=iota_t[:, None, :].to_broadcast([128, NSUB, 128]),
            in1=dstl[:, :, None].to_broadcast([128, NSUB, 128]),
            op=OP.is_equal,
        )
        # transposed one-hots (for d expansion), via PE transpose in groups of 4
        Utf = sb.tile([128, NSUBMAX, 128], bf16, tag="Ut", bufs=3)
        Ut = Utf[:, 0:NSUB, :]
        for g in range(0, NSUB, 4):
            n = min(4, NSUB - g)
            pst = ps.tile([128, 512], bf16, tag="pst", bufs=2)
            for kk in range(n):
                nc.tensor.transpose(
                    out=pst[:, kk * 128 : (kk + 1) * 128],
                    in_=U[:, g + kk, :],
                    identity=ident_t[:],
                )
            nc.scalar.activation(
                out=Ut[:, g : g + n, :],
                in_=pst[:, : n * 128].rearrange("p (j e) -> p j e", j=n),
                func=AF.Copy,
            )
        # d per edge: d_pe[e, h] = d_blk[dst_local[e], h]
        dblk = own[:, HF + H : HF + 2 * H]
        psd = ps.tile([128, NSUBMAX * H], f32, tag="psdpe", bufs=2)
        for j in range(NSUB):
            nc.tensor.matmul(
                out=psd[:, j * H : (j + 1) * H],
                lhsT=Ut[:, j, :],
                rhs=dblk,
                start=True,
                stop=True,
            )
        # ex = exp(leaky_relu(s_src + d_dst)); Lrelu+Exp fused on ACT
        alpha = sb.tile([128, NSUBMAX * H], f32, tag="alpha", bufs=3)
        nc.vector.tensor_tensor(
            out=alpha[:, 0 : NSUB * H].rearrange("p (j h) -> p j h", j=NSUB),
            in0=G[:, 0:NSUB, HF : HF + H],
            in1=psd[:, 0 : NSUB * H].rearrange("p (j h) -> p j h", j=NSUB),
            op=OP.add,
        )
        lr = sb.tile([128, NSUBMAX * H], f32, tag="lr", bufs=3)
        nc.scalar.activation(
            out=lr[:, 0 : NSUB * H], in_=alpha[:, 0 : NSUB * H],
            func=AF.Lrelu, alpha=NEG,
        )
        # rhs tile for the psa accumulation: [xw*ex | ex]; exp lands directly
        # in the ex columns and the multiply re-reads them.
        gwf = sb.tile([128, NSUBMAX, rcols], bf16, tag="gw", bufs=3)
        gw = gwf[:, 0:NSUB, :]
        nc.scalar.activation(
            out=gw[:, :, HF:rcols],
            in_=lr[:, 0 : NSUB * H].rearrange("p (j h) -> p j h", j=NSUB),
            func=AF.Exp,
        )
        nc.vector.tensor_tensor(
            out=gw[:, :, 0:HF].rearrange("p j (h f) -> p j h f", h=H),
            in0=G[:, 0:NSUB, 0:HF].rearrange("p j (h f) -> p j h f", h=H),
            in1=gw[:, :, HF:rcols][:, :, :, None].to_broadcast(
                [128, NSUB, H, F]
            ),
            op=OP.mult,
        )
        # self-loop contribution from the local table rows
        alph_s = sb.tile([128, H], f32, tag="alphs", bufs=3)
        nc.vector.tensor_tensor(
            out=alph_s[:],
            in0=own[:, HF : HF + H],
            in1=own[:, HF + H : HF + 2 * H],
            op=OP.add,
        )
        lrs = sb.tile([128, H], f32, tag="lrs", bufs=3)
        nc.scalar.activation(out=lrs[:], in_=alph_s[:], func=AF.Lrelu, alpha=NEG)
        rs = sb.tile([128, rcols], bf16, tag="rs", bufs=3)
        nc.scalar.activation(out=rs[:, HF:rcols], in_=lrs[:], func=AF.Exp)
        nc.vector.tensor_tensor(
            out=rs[:, 0:HF].rearrange("p (h f) -> p h f", h=H),
            in0=own[:, 0:HF].rearrange("p (h f) -> p h f", h=H),
            in1=rs[:, HF:rcols][:, :, None].to_broadcast([128, H, F]),
            op=OP.mult,
        )
        # accumulate [num | den]: self-loop first, then the gathered sub-tiles
        psa = ps.tile([128, rcols], f32, tag="psa", bufs=2)
        nc.tensor.matmul(
            out=psa[:], lhsT=ident_t[:], rhs=rs[:], start=True, stop=False
        )
        for j in range(NSUB):
            nc.tensor.matmul(
                out=psa[:],
                lhsT=U[:, j, :],
                rhs=gw[:, j, :],
                start=False,
                stop=(j == NSUB - 1),
            )
        # normalize + affine + activation
        rden = sb.tile([128, H], f32, tag="rden", bufs=3)
        nc.vector.reciprocal_approx_fast(out=rden[:], in_=psa[:, HF:rcols])
        o1 = sb.tile([128, HF], f32, tag="o1", bufs=3)
        nc.vector.tensor_tensor(
            out=o1[:].rearrange("p (h f) -> p h f", h=H),
            in0=psa[:, 0:HF].rearrange("p (h f) -> p h f", h=H),
            in1=rden[:].to_broadcast([128, H, F]),
            op=OP.mult,
        )
        o3 = sb.tile([128, HF], f32, tag="o3", bufs=3)
        nc.vector.tensor_tensor(out=o3[:], in0=o1[:], in1=bn_c_t[:], op=OP.add)
        if final:
            outt = sb.tile([128, HF], f32, tag="outt", bufs=3)
            nc.scalar.activation(out=outt[:], in_=o3[:], func=AF.Sigmoid)
            nc.sync.dma_start(out=h_out[b * 128 : (b + 1) * 128, :], in_=outt[:])
        else:
            # elu(x) = max(x, exp(min(x, 0)) - 1); min(x,0) == -relu(-x)
            e1 = sb.tile([128, HF], f32, tag="e1", bufs=3)
            nc.scalar.activation(out=e1[:], in_=o3[:], func=AF.Relu, scale=-1.0)
            e2 = sb.tile([128, HF], f32, tag="e2", bufs=3)
            nc.scalar.activation(out=e2[:], in_=e1[:], func=AF.Exp, scale=-1.0)
            hb = sb.tile([128, HF], bf16, tag="hb", bufs=3)
            nc.vector.scalar_tensor_tensor(
                out=hb[:],
                in0=e2[:],
                scalar=-1.0,
                in1=o3[:],
                op0=OP.add,
                op1=OP.max,
            )
            nc.sync.dma_start(out=h_out[b * 128 : (b + 1) * 128, :], in_=hb[:])


def _build_program(nlo_l, nhi_l, ML, NSUBMAX):
    nc = bacc.Bacc(
        "TRN2",
        target_bir_lowering=False,
        debug=False,
        num_devices=NCORES,
        num_swdge_queues=4,
    )
    HD = HEADS * HID

    # --- inputs ---
    t_x = nc.dram_tensor("x_shard", [SHARD, DIN], bf16, kind="ExternalInput")
    t_w1 = nc.dram_tensor("W1p", [DIN, USED12], bf16, kind="ExternalInput")
    t_w2 = nc.dram_tensor("W2p", [HD, USED12], bf16, kind="ExternalInput")
    t_w3 = nc.dram_tensor("W3p", [HD, USED3], bf16, kind="ExternalInput")
    t_c1 = nc.dram_tensor("bn_c1", [128, HD], f32, kind="ExternalInput")
    t_c2 = nc.dram_tensor("bn_c2", [128, HD], f32, kind="ExternalInput")
    t_c3 = nc.dram_tensor("bn_c3", [128, DOUT], f32, kind="ExternalInput")
    t_iota = nc.dram_tensor("iota_bf", [128, 128], bf16, kind="ExternalInput")
    t_iotap = nc.dram_tensor("iota_p", [128, 1], bf16, kind="ExternalInput")
    t_meta = nc.dram_tensor("meta", [NBLK, 128, ML], i16, kind="ExternalInput")
    t_meta2 = nc.dram_tensor(
        "meta2", [NBLK, 1, NSUBMAX * 128], i16, kind="ExternalInput"
    )
    t_counts = nc.dram_tensor("counts", [1, NBLK * 2], i32, kind="ExternalInput")
    t_out = nc.dram_tensor("out_shard", [SHARD, DOUT], f32, kind="ExternalOutput")

    with tile.TileContext(nc) as tc:
        with (
            tc.tile_pool(name="sb", bufs=2) as sb,
            tc.tile_pool(name="ps", bufs=2, space="PSUM") as ps,
            tc.tile_pool(name="dram", bufs=1, space="DRAM") as dr,
        ):
            # DRAM intermediates (pool tiles so Tile tracks dependencies)
            tbf = {
                (lyr, k): dr.tile(
                    [NCORES * (CH0 if k == 0 else CH1), DROW12 if lyr < 3 else DROW3],
                    bf16,
                    addr_space="Shared",
                    name=f"tb{lyr}_full{k}",
                )
                for lyr in (1, 2, 3)
                for k in (0, 1)
            }
            tbc = {
                (lyr, k): dr.tile(
                    [CH0 if k == 0 else CH1, DROW12 if lyr < 3 else DROW3],
                    bf16,
                    name=f"tb{lyr}_c{k}",
                )
                for lyr in (1, 2, 3)
                for k in (0, 1)
            }
            h2_own = dr.tile([SHARD, HD], bf16, name="h2_own")
            h3_own = dr.tile([SHARD, HD], bf16, name="h3_own")

            # constants
            iota_t = sb.tile([128, 128], bf16, tag="iota", bufs=1)
            nc.sync.dma_start(out=iota_t[:], in_=t_iota[:])
            iota_p = sb.tile([128, 1], bf16, tag="iotap", bufs=1)
            nc.sync.dma_start(out=iota_p[:], in_=t_iotap[:])
            ident_t = sb.tile([128, 128], bf16, tag="ident", bufs=1)
            make_identity(nc, ident_t[:])
            counts_t = sb.tile([1, NBLK * 2], i32, tag="counts", bufs=1)
            nc.sync.dma_start(out=counts_t[:], in_=t_counts[:])
            regs = [nc.gpsimd.alloc_register(f"cnt_{i}") for i in range(NREGS)]
            w1t = [sb.tile([128, USED12], bf16, tag="w1", bufs=1, name="w1t0")]
            nc.sync.dma_start(out=w1t[0][:], in_=t_w1[:])
            w2t = [sb.tile([128, USED12], bf16, tag=f"w2_{k}", bufs=1, name=f"w2t{k}") for k in range(2)]
            for k in range(2):
                nc.sync.dma_start(out=w2t[k][:], in_=t_w2[k * 128 : (k + 1) * 128, :])
            w3t = [sb.tile([128, USED3], bf16, tag=f"w3_{k}", bufs=1, name=f"w3t{k}") for k in range(2)]
            for k in range(2):
                nc.sync.dma_start(out=w3t[k][:], in_=t_w3[k * 128 : (k + 1) * 128, :])
            bn = {}
            for nm, t, w in (("c1", t_c1, HD), ("c2", t_c2, HD), ("c3", t_c3, DOUT)):
                bt = sb.tile([128, w], f32, tag=f"bn{nm}", bufs=1, name=f"bn{nm}")
                nc.sync.dma_start(out=bt[:], in_=t[:])
                bn[nm] = bt
            gb12 = [
                sb.tile([128, NSUBMAX, DROW12], bf16, tag=f"G12_{i}", bufs=1, name=f"G12_{i}")
                for i in range(GBUFS)
            ]
            gb3 = [
                sb.tile([128, NSUBMAX, DROW3], bf16, tag=f"G3_{i}", bufs=1, name=f"G3_{i}")
                for i in range(GBUFS)
            ]
            for gt in gb12 + gb3:
                nc.vector.memset(gt[:], 0.0)

            rg = [list(range(NCORES))]

            def ag_chunk(lyr, k):
                # collectives hold the gpsimd engine until completion, so the
                # edge phase sandwiches the first lookahead lo-gathers between
                # chunk 0 and chunk 1 (their DMA overlaps chunk 1's wait).
                nc.gpsimd.collective_compute(
                    "AllGather", OP.bypass, replica_groups=rg,
                    ins=[tbc[(lyr, k)][:]], outs=[tbf[(lyr, k)][:]],
                )

            # ---- layer 1 ----
            _dense_phase(
                nc, tc, sb, ps, t_x, w1t, tbc[(1, 0)], tbc[(1, 1)],
                USED12, DIN,
            )
            ag_chunk(1, 0)
            _edge_phase(
                nc, tc, sb, ps, gb12, tbf[(1, 0)], tbf[(1, 1)],
                tbc[(1, 0)], tbc[(1, 1)], t_meta, t_meta2, counts_t, regs,
                iota_t, iota_p, ident_t, HEADS, HID, DROW12, USED12, bn["c1"],
                h2_own, False, nlo_l, nhi_l, NSUBMAX,
                ag_c1=lambda: ag_chunk(1, 1),
            )
            # ---- layer 2 ----
            _dense_phase(
                nc, tc, sb, ps, h2_own, w2t, tbc[(2, 0)], tbc[(2, 1)],
                USED12, HD,
            )
            ag_chunk(2, 0)
            _edge_phase(
                nc, tc, sb, ps, gb12, tbf[(2, 0)], tbf[(2, 1)],
                tbc[(2, 0)], tbc[(2, 1)], t_meta, t_meta2, counts_t, regs,
                iota_t, iota_p, ident_t, HEADS, HID, DROW12, USED12, bn["c2"],
                h3_own, False, nlo_l, nhi_l, NSUBMAX,
                ag_c1=lambda: ag_chunk(2, 1),
            )
            # ---- layer 3 ----
            _dense_phase(
                nc, tc, sb, ps, h3_own, w3t, tbc[(3, 0)], tbc[(3, 1)],
                USED3, HD,
            )
            ag_chunk(3, 0)
            _edge_phase(
                nc, tc, sb, ps, gb3, tbf[(3, 0)], tbf[(3, 1)],
                tbc[(3, 0)], tbc[(3, 1)], t_meta, t_meta2, counts_t, regs,
                iota_t, iota_p, ident_t, 1, DOUT, DROW3, USED3, bn["c3"],
                t_out, True, nlo_l, nhi_l, NSUBMAX,
                ag_c1=lambda: ag_chunk(3, 1),
            )

    nc.compile()
    return nc


_CACHED = {}


def kernel(**inputs):
    x = np.asarray(inputs["x"], np.float32)
    edge_src = np.asarray(inputs["edge_src"], np.int32)
    edge_dst = np.asarray(inputs["edge_dst"], np.int32)

    xp = np.zeros((NPAD, DIN), np.float32)
    xp[:N] = x
    xb = xp.astype(bfnp)

    def aff(g, v, b, m, be):
        a = np.asarray(g, np.float32) / np.sqrt(np.asarray(v, np.float32) + EPS)
        c = (np.asarray(b, np.float32) - np.asarray(m, np.float32)) * a + np.asarray(
            be, np.float32
        )
        return a, c

    a1, c1 = aff(inputs["g1"], inputs["v1"], inputs["b1"], inputs["m1"], inputs["be1"])
    a2, c2 = aff(inputs["g2"], inputs["v2"], inputs["b2"], inputs["m2"], inputs["be2"])
    a3 = np.ones(DOUT, np.float32)
    c3 = np.asarray(inputs["b3"], np.float32)

    W1p = _build_Wp(
        np.asarray(inputs["W1"], np.float32),
        np.asarray(inputs["as1"], np.float32),
        np.asarray(inputs["ad1"], np.float32),
        a1,
    )
    W2p = _build_Wp(
        np.asarray(inputs["W2"], np.float32),
        np.asarray(inputs["as2"], np.float32),
        np.asarray(inputs["ad2"], np.float32),
        a2,
    )
    W3p = _build_Wp(
        np.asarray(inputs["W3"], np.float32),
        np.asarray(inputs["as3"], np.float32),
        np.asarray(inputs["ad3"], np.float32),
        a3,
    )

    edata, nlo_l, nhi_l, ML, NSUBMAX = _prep_edges(edge_src, edge_dst)
    iota = np.tile(np.arange(128, dtype=np.float32), (128, 1)).astype(bfnp)
    iotap = np.arange(128, dtype=np.float32).reshape(128, 1).astype(bfnp)

    key = (tuple(nlo_l), tuple(nhi_l))
    if _CACHED.get("key") != key:
        _CACHED["nc"] = _build_program(nlo_l, nhi_l, ML, NSUBMAX)
        _CACHED["key"] = key
    nc = _CACHED["nc"]

    def bcast(v):
        return np.tile(np.asarray(v, np.float32), (128, 1))

    in_maps = []
    for c in range(NCORES):
        in_maps.append(
            {
                "x_shard": xb[c * SHARD : (c + 1) * SHARD],
                "W1p": W1p,
                "W2p": W2p,
                "W3p": W3p,
                "bn_c1": bcast(c1),
                "bn_c2": bcast(c2),
                "bn_c3": bcast(c3),
                "iota_bf": iota,
                "iota_p": iotap,
                "meta": edata[c]["meta"],
                "meta2": edata[c]["meta2"],
                "counts": edata[c]["counts"].reshape(1, -1),
            }
        )

    trace = bool(os.environ.get("GAT_TRACE"))
    res = run_bass_kernel_spmd(
        nc, in_maps, core_ids=list(range(NCORES)), trace=trace
    )
    if trace and res.exec_time_ns:
        print(f"HW exec time: {res.exec_time_ns} ns")
    out = np.concatenate([res.results[c]["out_shard"] for c in range(NCORES)], axis=0)
    return np.ascontiguousarray(out[:N]).astype(np.float32)
